# revision 40
# baseline (speedup 1.0000x reference)
"""AtomPoolingLayer Trainium2 kernel (8 NeuronCores, data-parallel over molecules).

Reference computation (per molecule m of 512, atoms n=128, features f=512):
    w = sigmoid(relu(h @ W1 + b1) @ W2 + b2)        # gate, [M, N, 1]
    out[m, f] = sum_n w[m, n] * h[m, n, f]          # weighted pool, [M, F]

Sharding: h split on molecule dim across 8 cores (64 molecules/core); the tiny
MLP weights are replicated. No collectives needed.

Per-core pipeline (bf16 matmuls, DMA-bound target ~47us/core):
  DMA h f32 (natural [atom, mol, F] layout) -> DVE cast bf16 -> PE transpose
  (identity trick, 2-molecule pairs per PSUM bank) -> DVE copy PSUM->SBUF hT ->
  PE stage1 zT = W1.T @ hT (one 512-col sweep per F-chunk) -> ACT relu(+b1) ->
  PE stage2 w = zr.T @ W2 -> ACT sigmoid(+b2) -> PE stage3 out_m = w_m.T @ h_m
  -> ACT copy -> out DMA on the ACT hardware queue.

h loads ride two serialized SP DMA chains (each dma_start spreads over the DMA
engine pool at ~233 GB/s; two concurrent chains saturate the ~358 GB/s core
cap) with small first pieces so group 0 lands ~3us in. Engine discipline: the
Matmult slot supports only ONE sync wait, so every matmul's cross-engine deps
resolve to a single semaphore tick: hT copies all on DVE (cast ticks subsumed),
relu/sigmoid on ACT observed one group ahead by the 3-deep software pipeline.
"""

import numpy as np

import concourse.bass as bass
import concourse.mybir as mybir
import concourse.tile as tile
from concourse.bass_utils import run_bass_kernel_spmd

M, N, F = 512, 128, 512
HID = 128
N_CORES = 8
M_PER_CORE = M // N_CORES  # 64
G = 4  # molecules per pipeline group
N_GROUPS = M_PER_CORE // G  # 16
OB_BLOCK = 8  # groups per output staging block / out-DMA
CAST_AHEAD = 2  # cast lookahead in groups (extra lookahead adds to the tail)
FP = mybir.dt.float32
BF = mybir.dt.bfloat16

_AF = mybir.ActivationFunctionType

_LAST_RESULTS = None


def build_bass():
    nc = bass.Bass()

    h_ext = nc.declare_dram_parameter("h", [M_PER_CORE, N, F], FP, isOutput=False)
    w1_ext = nc.declare_dram_parameter("W1", [F, HID], FP, isOutput=False)
    b1_ext = nc.declare_dram_parameter("b1", [HID], FP, isOutput=False)
    w2_ext = nc.declare_dram_parameter("W2", [HID, 1], FP, isOutput=False)
    b2_ext = nc.declare_dram_parameter("b2", [1], FP, isOutput=False)
    out_ext = nc.declare_dram_parameter("out", [M_PER_CORE, F], FP, isOutput=True)

    with tile.TileContext(nc) as tc:
        with (
            tc.tile_pool(name="singles", bufs=1) as singles,
            tc.tile_pool(name="hf32", bufs=4) as hf32p,
            tc.tile_pool(name="hbf", bufs=8) as hbfp,
            tc.tile_pool(name="ht", bufs=3) as htp,
            tc.tile_pool(name="zr", bufs=3) as zrp,
            tc.tile_pool(name="ps_t", bufs=3, space="PSUM") as pstp,
            tc.tile_pool(name="ps_z", bufs=2, space="PSUM") as pszp,
            tc.tile_pool(name="ps_w", bufs=1, space="PSUM") as pswp,
            tc.tile_pool(name="ps_o", bufs=2, space="PSUM") as psop,
        ):
            from concourse.bass import _add_dep_helper

            chains = {}

            def chained(key, inst):
                prev = chains.get(key)
                if prev is not None:
                    _add_dep_helper(
                        inst.ins, prev.ins, sync=False, reason=f"{key} order"
                    )
                chains[key] = inst
                return inst

            def pe(inst):
                return chained("pe", inst)

            def act(inst):
                return chained("act", inst)

            def dve(inst):
                return chained("dve", inst)

            def probe(chain_key, inst, dep):
                chained(chain_key, inst)
                _add_dep_helper(inst.ins, dep.ins, sync=True, reason="probe")
                return inst

            # ---------------- constants ----------------
            # identity: f32 build on (idle) gpsimd, bf16 round on DVE.
            # Chained into the "pool" order so the later out-DMA triggers
            # (also on Pool) cannot be scheduled ahead of the build.
            ident_f32 = singles.tile([128, 128], FP)
            chained("pool", nc.gpsimd.memset(ident_f32, 0.0))
            ident_mk = chained(
                "pool",
                nc.gpsimd.affine_select(
                    out=ident_f32,
                    in_=ident_f32,
                    compare_op=mybir.AluOpType.not_equal,
                    fill=1.0,
                    base=0,
                    pattern=[[-1, 128]],
                    channel_multiplier=1,
                ),
            )
            ident = singles.tile([128, 128], BF)
            # (bf16 round of the identity is issued on DVE after the primed
            # casts, so early casts aren't queued behind the Pool build)

            # The first two h pieces go out BEFORE the constants: the
            # constant loads are tiny-descriptor DMAs (W1 is 512x512B, the
            # bias loads 128x4B) that clog the DMA rings for several us at
            # poor byte-rate, and ring order is issue order.
            h_view = h_ext[:]  # [M_PER_CORE, N, F]
            PIECES = [(s, s + 4) for s in range(0, 64, 4)]
            hf_tiles = [None] * 4  # 16-molecule f32 staging tiles
            all_load_dmas = []

            def issue_piece(lo, hi):
                L = lo // 16
                if hf_tiles[L] is None:
                    hf_tiles[L] = hf32p.tile(
                        [128, 16, F], FP, name=f"hf{L}", tag="hf"
                    )
                dma = chained(
                    "sp",
                    nc.sync.dma_start(
                        out=hf_tiles[L][:, lo - 16 * L : hi - 16 * L, :],
                        in_=h_view[lo:hi].rearrange("g n f -> n g f"),
                    ),
                )
                all_load_dmas.append(dma)

            for lo, hi in PIECES[:2]:
                issue_piece(lo, hi)

            # Constants on HWDGE next: their completion semaphores recycle
            # harmlessly (the SP nop pacing below keeps every piece's
            # implicit recycle wait pre-satisfied). The consuming engine
            # absorbs each completion tick so users carry no DMA waits.
            # W1 [F, HID] -> SBUF [k=128 (F within chunk), c=4 (F chunk), HID]
            w1f = singles.tile([128, 4, HID], FP)
            cdma1 = chained("sp", nc.sync.dma_start(
                out=w1f, in_=w1_ext[:].rearrange("(c k) h -> k c h", k=128)
            ))
            w1b = singles.tile([128, 4, HID], BF)

            # b1 [HID] -> [128, 1] f32, absorbed through ACT (its consumer)
            b1raw = singles.tile([128, 1], FP)
            cdma2 = chained("sp", nc.sync.dma_start(
                out=b1raw, in_=b1_ext[:].rearrange("(p o) -> p o", o=1)
            ))
            b1s = singles.tile([128, 1], FP)
            act(nc.scalar.copy(b1s, b1raw))

            # W2 [HID, 1] -> bf16 [128, 1], absorbed on ACT
            w2f = singles.tile([128, 1], FP)
            cdma3 = chained("sp", nc.sync.dma_start(out=w2f, in_=w2_ext[:]))
            w2b = singles.tile([128, 1], BF)
            act(nc.scalar.copy(w2b, w2f))

            # b2 [1] broadcast -> [128, 1] f32, absorbed through ACT
            b2raw = singles.tile([128, 1], FP)
            b2_bcast = bass.AP(tensor=b2_ext, offset=0, ap=[[0, 128], [1, 1]])
            cdma4 = chained("sp", nc.sync.dma_start(out=b2raw, in_=b2_bcast))
            b2s = singles.tile([128, 1], FP)
            act(nc.scalar.copy(b2s, b2raw))

            # gate weights accumulate here: [atom, molecule] bf16
            w_sig = singles.tile([128, M_PER_CORE], BF)
            psum_w = pswp.tile([128, M_PER_CORE], FP)

            # output staging: molecule j of each group lands on partition 32j
            ob4 = singles.tile([128, OB_BLOCK, F], FP)

            # probe scratch (disjoint columns -> no probe-to-probe deps).
            # The tile framework enforces same-engine pipeline hazards with
            # SELF-semaphore waits (e.g. sigmoid waits relu COMPLETE); when an
            # instruction also has a cross-engine dep that makes 2 waits,
            # which exceeds the 1-wait ISA slot. Per-iteration probes absorb
            # the self-ticks on dedicated cheap instructions instead.
            scr_act = singles.tile([1, N_GROUPS + 4], FP)
            scr_ob = singles.tile([1, 2 * (N_GROUPS // OB_BLOCK) + 2], FP)
            scr_dve = singles.tile([1, N_GROUPS + 4], FP)

            # one-time ACT probe past the constant copies: absorbs the
            # b1s/w2b/b2s completion ticks so the first relu's bias-RAW
            # self-wait is already observed
            act(nc.scalar.copy(scr_act[0:1, N_GROUPS + 3 :], b2s[0:1, :]))

            # ---------------- h load: SP-nop paced piece stream ----------
            # Each dma_start spreads over the DMA engine pool; ~2 concurrent
            # pieces saturate HBM. Delivery must be in molecule order
            # (unordered round-robin draining would make everything finish at
            # once, starving the pipeline head), and each piece trigger may
            # carry at most ONE sync wait — which the framework already uses
            # for completion-semaphore recycling past the 8th HWDGE DMA. So
            # pacing lives on SP sequencer nops instead: a nop waits piece
            # k-2's completion, then piece k's trigger issues; with 2 pieces
            # in flight, every implicit recycle wait is long satisfied.
            # Pieces never straddle cast boundaries (casts carry one DMA
            # wait each).
            for idx, (lo, hi) in enumerate(PIECES):
                if idx < 2:
                    continue  # issued before the constants
                if idx >= 3:
                    # depth-3: piece k issues when k-3 completes, keeping 3
                    # pieces in flight so trigger latency never drains HBM
                    probe(
                        "sp",
                        nc.sync.nop(nofuse=True, hint="load_pace"),
                        all_load_dmas[idx - 3],
                    )
                issue_piece(lo, hi)

            # group g is covered by piece g: plain single-wait casts
            assert len(PIECES) == N_GROUPS

            # ---------------- per-group state ----------------
            hb_tiles = [None] * N_GROUPS
            ht_tiles = [None] * N_GROUPS
            zr_tiles = [None] * N_GROUPS
            s3_last = [None] * N_GROUPS
            obcopy_last = [None] * N_GROUPS
            outdma = [None] * (N_GROUPS // OB_BLOCK)

            def issue_cast(g):
                hb = hbfp.tile([128, G, F], BF, name=f"hb{g}", tag="hb")
                hb_tiles[g] = hb
                L, gi = g // 4, g % 4
                src = hf_tiles[L][:, gi * G : (gi + 1) * G, :]
                dve(nc.vector.tensor_copy(hb, src))

            def front_transposes(g):
                # transposes (2-molecule pairs) + hT PSUM->SBUF copies
                hb = hb_tiles[g]
                ht = htp.tile([128, 4, G, 128], BF, name=f"ht{g}", tag="ht")
                ht_tiles[g] = ht
                for p in range(2):  # molecule pairs (2p, 2p+1)
                    ps_pair = pstp.tile([128, 4, 2, 128], BF)
                    for jj in range(2):
                        j = 2 * p + jj
                        for c in range(4):
                            pe(
                                nc.tensor.transpose(
                                    ps_pair[:, c, jj, :],
                                    hb[:, j, c * 128 : (c + 1) * 128],
                                    ident,
                                )
                            )
                    dve(
                        nc.vector.tensor_copy(
                            ht[:, :, 2 * p : 2 * p + 2, :], ps_pair
                        )
                    )

            def front_stage1(g):
                # stage 1 (issued after mid/back of older groups so the DVE
                # hT copies have time to land) + relu
                ht = ht_tiles[g]
                ps_z = pszp.tile([128, G * 128], FP)
                for c in range(4):
                    pe(
                        nc.tensor.matmul(
                            ps_z,
                            w1b[:, c, :],
                            ht[:, c, :, :],
                            start=(c == 0),
                            stop=(c == 3),
                        )
                    )
                zr = zrp.tile([128, G * 128], BF, name=f"zr{g}", tag="zr")
                zr_tiles[g] = zr
                act(nc.scalar.activation(zr, ps_z, _AF.Relu, bias=b1s))

            def mid_stage(g):
                # stage 2 + sigmoid for group g
                zr = zr_tiles[g]
                for j in range(G):
                    mm = g * G + j
                    pe(
                        nc.tensor.matmul(
                            psum_w[:, mm : mm + 1],
                            zr[:, j * 128 : (j + 1) * 128],
                            w2b,
                            start=True,
                            stop=True,
                        )
                    )
                act(
                    nc.scalar.activation(
                        w_sig[:, g * G : (g + 1) * G],
                        psum_w[:, g * G : (g + 1) * G],
                        _AF.Sigmoid,
                        bias=b2s,
                    )
                )

            def back(g):
                # stage 3 + out staging + block DMA (on ACT queue) for group g
                hb = hb_tiles[g]
                if g % OB_BLOCK == 0 and g >= OB_BLOCK:
                    blk = g // OB_BLOCK
                    probe(
                        "act",
                        nc.scalar.mul(
                            scr_ob[0:1, blk : blk + 1],
                            scr_ob[0:1, blk : blk + 1],
                            0.0,
                        ),
                        outdma[blk - 1],
                    )
                ps_o = psop.tile([128, F], FP)
                for j in range(G):
                    mm = g * G + j
                    s3_last[g] = pe(
                        nc.tensor.matmul(
                            ps_o[32 * j : 32 * j + 1, :],
                            w_sig[:, mm : mm + 1],
                            hb[:, j, :],
                            start=True,
                            stop=True,
                            tile_position=(0, 32 * j),
                        )
                    )
                obcopy_last[g] = act(
                    nc.scalar.copy(ob4[:, g % OB_BLOCK, :], ps_o)
                )
                if g % OB_BLOCK == OB_BLOCK - 1:
                    blk = g // OB_BLOCK
                    # out-DMA on SWDGE: HWDGE sems stay dedicated to h loads;
                    # the SWDGE sem recycle (constants) is absorbed by the
                    # early pool probes, leaving one ACT wait on the trigger
                    outdma[blk] = chained(
                        "pool",
                        nc.gpsimd.dma_start(
                            out=out_ext[
                                blk * OB_BLOCK * G : (blk + 1) * OB_BLOCK * G
                            ].rearrange("(gi j) f -> j gi f", j=G),
                            in_=ob4[0:128:32, :, :],
                        ),
                    )

            # prime casts first on DVE (they only need the first h pieces),
            # then the identity round (Pool build lands ~7us) and last the
            # W1 round (the W1 DMA is the slowest-arriving dependency)
            for g in range(CAST_AHEAD):
                issue_cast(g)
            dve(nc.vector.tensor_copy(ident, ident_f32))
            dve(nc.vector.tensor_copy(w1b, w1f))

            # depth-4 software pipeline: S1(g-1) | T(g) | S2(g-2) | S3(g-3).
            # Every PE instruction's cross-engine dependency is at least one
            # iteration old, so PE never stalls on a fresh semaphore in
            # steady state; ACT trails PE within the iteration.
            act_iter_last = None
            dve_iter_last = None
            for it in range(N_GROUPS + 3):
                g_t, g_1, g_2, g_3 = it, it - 1, it - 2, it - 3
                # self-tick probes: absorb the previous iteration's ACT/DVE
                # completions so this iteration's instructions need only one
                # cross-engine wait each
                if act_iter_last is not None:
                    probe(
                        "act",
                        nc.scalar.mul(
                            scr_act[0:1, it : it + 1],
                            scr_act[0:1, it : it + 1],
                            0.0,
                        ),
                        act_iter_last,
                    )
                if dve_iter_last is not None:
                    probe(
                        "dve",
                        nc.vector.memset(scr_dve[0:1, it : it + 1], 0.0),
                        dve_iter_last,
                    )
                # cast first on DVE so the pair copies finish well before the
                # next iteration's stage1 needs them
                if g_t + CAST_AHEAD < N_GROUPS:
                    issue_cast(g_t + CAST_AHEAD)
                if 0 <= g_1 < N_GROUPS:
                    front_stage1(g_1)
                if g_t < N_GROUPS:
                    front_transposes(g_t)
                if 0 <= g_2 < N_GROUPS:
                    mid_stage(g_2)
                if 0 <= g_3 < N_GROUPS:
                    back(g_3)
                act_iter_last = chains.get("act")
                dve_iter_last = chains.get("dve")

            # ---- tail: pre-advance SP's observed ticks so Tile's final drain
            # needs no waits of its own
            tail_deps = []
            tail_deps.extend(all_load_dmas)
            tail_deps.extend(outdma)
            tail_deps.extend([cdma1, cdma2, cdma3, cdma4])
            tail_deps.append(ident_mk)  # Pool
            tail_deps.append(chains["dve"])  # DVE
            tail_deps.append(chains["act"])  # ACT
            tail_deps.append(chains["pe"])  # PE
            for dep in tail_deps:
                probe("sp", nc.sync.nop(nofuse=True, hint="tail_sink"), dep)

    return nc


_NC_CACHE = None


def kernel(h, W1, b1, W2, b2, _trace=False):
    global _NC_CACHE, _LAST_RESULTS
    h = np.ascontiguousarray(np.asarray(h, dtype=np.float32))
    W1 = np.ascontiguousarray(np.asarray(W1, dtype=np.float32))
    b1 = np.ascontiguousarray(np.asarray(b1, dtype=np.float32))
    W2 = np.ascontiguousarray(np.asarray(W2, dtype=np.float32))
    b2 = np.ascontiguousarray(np.asarray(b2, dtype=np.float32))

    if _NC_CACHE is None:
        _NC_CACHE = build_bass()
    nc = _NC_CACHE

    in_maps = []
    for i in range(N_CORES):
        in_maps.append(
            {
                "h": h[i * M_PER_CORE : (i + 1) * M_PER_CORE],
                "W1": W1,
                "b1": b1,
                "W2": W2,
                "b2": b2,
            }
        )

    res = run_bass_kernel_spmd(
        nc, in_maps, core_ids=list(range(N_CORES)), trace=_trace
    )
    _LAST_RESULTS = res
    out = np.concatenate([np.asarray(r["out"]) for r in res.results], axis=0)
    return out


# revision 55
# speedup vs baseline: 1.0472x; 1.0472x over previous
"""AtomPoolingLayer Trainium2 kernel (8 NeuronCores, data-parallel over molecules).

Reference computation (per molecule m of 512, atoms n=128, features f=512):
    w = sigmoid(relu(h @ W1 + b1) @ W2 + b2)        # gate, [M, N, 1]
    out[m, f] = sum_n w[m, n] * h[m, n, f]          # weighted pool, [M, F]

Sharding: h split on molecule dim across 8 cores (64 molecules/core); the tiny
MLP weights are replicated. No collectives needed.

Per-core pipeline (bf16 matmuls, DMA-bound target ~47us/core):
  DMA h f32 (natural [atom, mol, F] layout) -> DVE cast bf16 -> PE transpose
  (identity trick, 2-molecule pairs per PSUM bank) -> DVE copy PSUM->SBUF hT ->
  PE stage1 zT = W1.T @ hT (one 512-col sweep per F-chunk) -> ACT relu(+b1) ->
  PE stage2 w = zr.T @ W2 -> ACT sigmoid(+b2) -> PE stage3 out_m = w_m.T @ h_m
  -> ACT copy -> out DMA on the ACT hardware queue.

h loads ride two serialized SP DMA chains (each dma_start spreads over the DMA
engine pool at ~233 GB/s; two concurrent chains saturate the ~358 GB/s core
cap) with small first pieces so group 0 lands ~3us in. Engine discipline: the
Matmult slot supports only ONE sync wait, so every matmul's cross-engine deps
resolve to a single semaphore tick: hT copies all on DVE (cast ticks subsumed),
relu/sigmoid on ACT observed one group ahead by the 3-deep software pipeline.
"""

import numpy as np

import concourse.bass as bass
import concourse.mybir as mybir
import concourse.tile as tile
from concourse.bass_utils import run_bass_kernel_spmd

M, N, F = 512, 128, 512
HID = 128
N_CORES = 8
M_PER_CORE = M // N_CORES  # 64
G = 4  # molecules per pipeline group
N_GROUPS = M_PER_CORE // G  # 16
OB_BLOCK = 8  # groups per output staging block / out-DMA
CAST_AHEAD = 1  # cast lookahead in groups (extra lookahead adds to the tail)
FP = mybir.dt.float32
BF = mybir.dt.bfloat16

_AF = mybir.ActivationFunctionType

_LAST_RESULTS = None


def build_bass():
    nc = bass.Bass()

    h_ext = nc.declare_dram_parameter("h", [M_PER_CORE, N, F], FP, isOutput=False)
    w1_ext = nc.declare_dram_parameter("W1", [F, HID], FP, isOutput=False)
    b1_ext = nc.declare_dram_parameter("b1", [HID], FP, isOutput=False)
    w2_ext = nc.declare_dram_parameter("W2", [HID, 1], FP, isOutput=False)
    b2_ext = nc.declare_dram_parameter("b2", [1], FP, isOutput=False)
    out_ext = nc.declare_dram_parameter("out", [M_PER_CORE, F], FP, isOutput=True)

    with tile.TileContext(nc) as tc:
        with (
            tc.tile_pool(name="singles", bufs=1) as singles,
            tc.tile_pool(name="hf32", bufs=4) as hf32p,
            tc.tile_pool(name="hbf", bufs=8) as hbfp,
            tc.tile_pool(name="ht", bufs=3) as htp,
            tc.tile_pool(name="zr", bufs=3) as zrp,
            tc.tile_pool(name="ps_t", bufs=2, space="PSUM") as pstp,
            tc.tile_pool(name="ps_z", bufs=3, space="PSUM") as pszp,
            tc.tile_pool(name="ps_w", bufs=1, space="PSUM") as pswp,
            tc.tile_pool(name="ps_o", bufs=2, space="PSUM") as psop,
        ):
            from concourse.bass import _add_dep_helper

            chains = {}

            def chained(key, inst):
                prev = chains.get(key)
                if prev is not None:
                    _add_dep_helper(
                        inst.ins, prev.ins, sync=False, reason=f"{key} order"
                    )
                chains[key] = inst
                return inst

            def pe(inst):
                return chained("pe", inst)

            def act(inst):
                return chained("act", inst)

            def dve(inst):
                return chained("dve", inst)

            def probe(chain_key, inst, dep):
                chained(chain_key, inst)
                _add_dep_helper(inst.ins, dep.ins, sync=True, reason="probe")
                return inst

            # ---------------- constants ----------------
            # identity: f32 build on (idle) gpsimd, bf16 round on DVE.
            # Chained into the "pool" order so the later out-DMA triggers
            # (also on Pool) cannot be scheduled ahead of the build.
            ident_f32 = singles.tile([128, 128], FP)
            chained("pool", nc.gpsimd.memset(ident_f32, 0.0))
            ident_mk = chained(
                "pool",
                nc.gpsimd.affine_select(
                    out=ident_f32,
                    in_=ident_f32,
                    compare_op=mybir.AluOpType.not_equal,
                    fill=1.0,
                    base=0,
                    pattern=[[-1, 128]],
                    channel_multiplier=1,
                ),
            )
            ident = singles.tile([128, 128], BF)
            # (bf16 round of the identity is issued on DVE after the primed
            # casts, so early casts aren't queued behind the Pool build)

            # The first two h pieces go out BEFORE the constants: the
            # constant loads are tiny-descriptor DMAs (W1 is 512x512B, the
            # bias loads 128x4B) that clog the DMA rings for several us at
            # poor byte-rate, and ring order is issue order.
            h_view = h_ext[:]  # [M_PER_CORE, N, F]
            PIECES = [(s, s + 4) for s in range(0, 64, 4)]
            hf_tiles = [None] * 4  # 16-molecule f32 staging tiles
            all_load_dmas = []

            def issue_piece(lo, hi):
                L = lo // 16
                if hf_tiles[L] is None:
                    hf_tiles[L] = hf32p.tile(
                        [128, 16, F], FP, name=f"hf{L}", tag="hf"
                    )
                dma = chained(
                    "sp",
                    nc.sync.dma_start(
                        out=hf_tiles[L][:, lo - 16 * L : hi - 16 * L, :],
                        in_=h_view[lo:hi].rearrange("g n f -> n g f"),
                    ),
                )
                all_load_dmas.append(dma)

            for lo, hi in PIECES[:2]:
                issue_piece(lo, hi)

            # Constants on HWDGE next: their completion semaphores recycle
            # harmlessly (the SP nop pacing below keeps every piece's
            # implicit recycle wait pre-satisfied). The consuming engine
            # absorbs each completion tick so users carry no DMA waits.
            # W1 [F, HID] -> SBUF [k=128 (F within chunk), c=4 (F chunk), HID]
            w1f = singles.tile([128, 4, HID], FP)
            cdma1 = chained("sp", nc.sync.dma_start(
                out=w1f, in_=w1_ext[:].rearrange("(c k) h -> k c h", k=128)
            ))
            w1b = singles.tile([128, 4, HID], BF)

            # b1 [HID] -> [128, 1] f32, absorbed through ACT (its consumer)
            b1raw = singles.tile([128, 1], FP)
            cdma2 = chained("sp", nc.sync.dma_start(
                out=b1raw, in_=b1_ext[:].rearrange("(p o) -> p o", o=1)
            ))
            b1s = singles.tile([128, 1], FP)
            act(nc.scalar.copy(b1s, b1raw))

            # W2 [HID, 1] -> bf16 [128, 1], absorbed on ACT
            w2f = singles.tile([128, 1], FP)
            cdma3 = chained("sp", nc.sync.dma_start(out=w2f, in_=w2_ext[:]))
            w2b = singles.tile([128, 1], BF)
            act(nc.scalar.copy(w2b, w2f))

            # b2 [1] broadcast -> [128, 1] f32, absorbed through ACT
            b2raw = singles.tile([128, 1], FP)
            b2_bcast = bass.AP(tensor=b2_ext, offset=0, ap=[[0, 128], [1, 1]])
            cdma4 = chained("sp", nc.sync.dma_start(out=b2raw, in_=b2_bcast))
            b2s = singles.tile([128, 1], FP)
            act(nc.scalar.copy(b2s, b2raw))

            # gate weights accumulate here: [atom, molecule] bf16
            w_sig = singles.tile([128, M_PER_CORE], BF)
            psum_w = pswp.tile([128, M_PER_CORE], FP)

            # output staging: molecule j of each group lands on partition 32j
            ob4 = singles.tile([128, OB_BLOCK, F], FP)

            # PE-probe operand scratch: written once by DVE (an ancient tick
            # subsumed by every probe's explicit hT-copy wait)
            scr_pe = singles.tile([1, 1], FP)
            dve(nc.vector.memset(scr_pe, 0.0))

            # probe scratch (disjoint columns -> no probe-to-probe deps).
            # The tile framework enforces same-engine pipeline hazards with
            # SELF-semaphore waits (e.g. sigmoid waits relu COMPLETE); when an
            # instruction also has a cross-engine dep that makes 2 waits,
            # which exceeds the 1-wait ISA slot. Per-iteration probes absorb
            # the self-ticks on dedicated cheap instructions instead.
            scr_act = singles.tile([1, N_GROUPS + 4], FP)
            scr_ob = singles.tile([1, 2 * (N_GROUPS // OB_BLOCK) + 2], FP)
            scr_dve = singles.tile([1, N_GROUPS + 4], FP)

            # one-time ACT probe past the constant copies: absorbs the
            # b1s/w2b/b2s completion ticks so the first relu's bias-RAW
            # self-wait is already observed
            act(nc.scalar.copy(scr_act[0:1, N_GROUPS + 3 :], b2s[0:1, :]))

            # ---------------- h load: SP-nop paced piece stream ----------
            # Each dma_start spreads over the DMA engine pool; ~2 concurrent
            # pieces saturate HBM. Delivery must be in molecule order
            # (unordered round-robin draining would make everything finish at
            # once, starving the pipeline head), and each piece trigger may
            # carry at most ONE sync wait — which the framework already uses
            # for completion-semaphore recycling past the 8th HWDGE DMA. So
            # pacing lives on SP sequencer nops instead: a nop waits piece
            # k-2's completion, then piece k's trigger issues; with 2 pieces
            # in flight, every implicit recycle wait is long satisfied.
            # Pieces never straddle cast boundaries (casts carry one DMA
            # wait each).
            for idx, (lo, hi) in enumerate(PIECES):
                if idx < 2:
                    continue  # issued before the constants
                if idx >= 3:
                    # depth-3: piece k issues when k-3 completes, keeping 3
                    # pieces in flight so trigger latency never drains HBM
                    probe(
                        "sp",
                        nc.sync.nop(nofuse=True, hint="load_pace"),
                        all_load_dmas[idx - 3],
                    )
                issue_piece(lo, hi)

            # group g is covered by piece g: plain single-wait casts
            assert len(PIECES) == N_GROUPS

            # ---------------- per-group state ----------------
            hb_tiles = [None] * N_GROUPS
            ht_tiles = [None] * N_GROUPS
            zr_tiles = [None] * N_GROUPS
            s3_last = [None] * N_GROUPS
            obcopy_last = [None] * N_GROUPS
            outdma = [None] * (N_GROUPS // OB_BLOCK)

            def issue_cast(g):
                hb = hbfp.tile([128, G, F], BF, name=f"hb{g}", tag="hb")
                hb_tiles[g] = hb
                L, gi = g // 4, g % 4
                src = hf_tiles[L][:, gi * G : (gi + 1) * G, :]
                dve(nc.vector.tensor_copy(hb, src))

            copy_last = [None] * N_GROUPS

            def front_transposes(g):
                # transposes (2-molecule pairs per PSUM bank) + pair hT
                # PSUM->SBUF copies
                hb = hb_tiles[g]
                ht = htp.tile([128, 4, G, 128], BF, name=f"ht{g}", tag="ht")
                ht_tiles[g] = ht
                for p in range(2):  # molecule pairs (2p, 2p+1)
                    ps_pair = pstp.tile([128, 4, 2, 128], BF)
                    for jj in range(2):
                        j = 2 * p + jj
                        for c in range(4):
                            pe(
                                nc.tensor.transpose(
                                    ps_pair[:, c, jj, :],
                                    hb[:, j, c * 128 : (c + 1) * 128],
                                    ident,
                                )
                            )
                    copy_last[g] = dve(
                        nc.vector.tensor_copy(
                            ht[:, :, 2 * p : 2 * p + 2, :], ps_pair
                        )
                    )

            def front_stage1(g, ps_z):
                # stage 1 + relu (ps_z allocated at the iteration top so the
                # PE probe can write its corner first)
                ht = ht_tiles[g]
                for c in range(4):
                    pe(
                        nc.tensor.matmul(
                            ps_z,
                            w1b[:, c, :],
                            ht[:, c, :, :],
                            start=(c == 0),
                            stop=(c == 3),
                        )
                    )
                zr = zrp.tile([128, G * 128], BF, name=f"zr{g}", tag="zr")
                zr_tiles[g] = zr
                act(nc.scalar.activation(zr, ps_z, _AF.Relu, bias=b1s))

            def mid_stage(g):
                # stage 2 + sigmoid for group g
                zr = zr_tiles[g]
                for j in range(G):
                    mm = g * G + j
                    pe(
                        nc.tensor.matmul(
                            psum_w[:, mm : mm + 1],
                            zr[:, j * 128 : (j + 1) * 128],
                            w2b,
                            start=True,
                            stop=True,
                        )
                    )
                act(
                    nc.scalar.activation(
                        w_sig[:, g * G : (g + 1) * G],
                        psum_w[:, g * G : (g + 1) * G],
                        _AF.Sigmoid,
                        bias=b2s,
                    )
                )

            def back(g):
                # stage 3 + out staging + block DMA (on ACT queue) for group g
                hb = hb_tiles[g]
                if g % OB_BLOCK == 0 and g >= OB_BLOCK:
                    blk = g // OB_BLOCK
                    probe(
                        "act",
                        nc.scalar.mul(
                            scr_ob[0:1, blk : blk + 1],
                            scr_ob[0:1, blk : blk + 1],
                            0.0,
                        ),
                        outdma[blk - 1],
                    )
                ps_o = psop.tile([128, F], FP)
                for j in range(G):
                    mm = g * G + j
                    s3_last[g] = pe(
                        nc.tensor.matmul(
                            ps_o[32 * j : 32 * j + 1, :],
                            w_sig[:, mm : mm + 1],
                            hb[:, j, :],
                            start=True,
                            stop=True,
                            tile_position=(0, 32 * j),
                        )
                    )
                obcopy_last[g] = act(
                    nc.scalar.copy(ob4[:, g % OB_BLOCK, :], ps_o)
                )
                if g % OB_BLOCK == OB_BLOCK - 1:
                    blk = g // OB_BLOCK
                    # out-DMA on SWDGE: HWDGE sems stay dedicated to h loads;
                    # the SWDGE sem recycle (constants) is absorbed by the
                    # early pool probes, leaving one ACT wait on the trigger
                    outdma[blk] = chained(
                        "pool",
                        nc.gpsimd.dma_start(
                            out=out_ext[
                                blk * OB_BLOCK * G : (blk + 1) * OB_BLOCK * G
                            ].rearrange("(gi j) f -> j gi f", j=G),
                            in_=ob4[0:128:32, :, :],
                        ),
                    )

            # prime casts first on DVE (they only need the first h pieces),
            # then the identity round (Pool build lands ~7us) and last the
            # W1 round (the W1 DMA is the slowest-arriving dependency)
            for g in range(CAST_AHEAD):
                issue_cast(g)
            dve(nc.vector.tensor_copy(ident, ident_f32))
            dve(nc.vector.tensor_copy(w1b, w1f))

            # depth-4 software pipeline: S1(g-1) | T(g) | S2(g-2) | S3(g-3).
            # Every PE instruction's cross-engine dependency is at least one
            # iteration old, so PE never stalls on a fresh semaphore in
            # steady state; ACT trails PE within the iteration.
            act_iter_last = None
            dve_iter_last = None
            for it in range(N_GROUPS + 3):
                g_t, g_1, g_2, g_3 = it, it - 1, it - 2, it - 3
                # self-tick probes: absorb the previous iteration's ACT/DVE
                # completions so this iteration's instructions need only one
                # cross-engine wait each
                if act_iter_last is not None:
                    probe(
                        "act",
                        nc.scalar.mul(
                            scr_act[0:1, it : it + 1],
                            scr_act[0:1, it : it + 1],
                            0.0,
                        ),
                        act_iter_last,
                    )
                if dve_iter_last is not None:
                    probe(
                        "dve",
                        nc.vector.memset(scr_dve[0:1, it : it + 1], 0.0),
                        dve_iter_last,
                    )
                # cast first on DVE so the pair copies finish early
                if g_t + CAST_AHEAD < N_GROUPS:
                    issue_cast(g_t + CAST_AHEAD)
                ps_z = None
                if 0 <= g_1 < N_GROUPS:
                    ps_z = pszp.tile([128, G * 128], FP)
                    # PE probe: a 1x1 matmul into this iteration's ps_z
                    # corner (stage1's start=True overwrites it), reading
                    # never-written scratch. Its ONE wait absorbs the
                    # previous group's last hT-copy tick, so the transposes
                    # and stage1 below carry at most one new tick each.
                    probe(
                        "pe",
                        nc.tensor.matmul(
                            ps_z[0:1, 0:1],
                            scr_pe,
                            scr_pe,
                            start=True,
                            stop=True,
                        ),
                        copy_last[g_1],
                    )
                if g_t < N_GROUPS:
                    front_transposes(g_t)
                if 0 <= g_1 < N_GROUPS:
                    front_stage1(g_1, ps_z)
                if 0 <= g_2 < N_GROUPS:
                    mid_stage(g_2)
                if 0 <= g_3 < N_GROUPS:
                    back(g_3)
                act_iter_last = chains.get("act")
                dve_iter_last = chains.get("dve")

            # ---- tail: pre-advance SP's observed ticks so Tile's final drain
            # needs no waits of its own
            tail_deps = []
            tail_deps.extend(all_load_dmas)
            tail_deps.extend(outdma)
            tail_deps.extend([cdma1, cdma2, cdma3, cdma4])
            tail_deps.append(ident_mk)  # Pool
            tail_deps.append(chains["dve"])  # DVE
            tail_deps.append(chains["act"])  # ACT
            tail_deps.append(chains["pe"])  # PE
            for dep in tail_deps:
                probe("sp", nc.sync.nop(nofuse=True, hint="tail_sink"), dep)

    return nc


_NC_CACHE = None


def kernel(h, W1, b1, W2, b2, _trace=False):
    global _NC_CACHE, _LAST_RESULTS
    h = np.ascontiguousarray(np.asarray(h, dtype=np.float32))
    W1 = np.ascontiguousarray(np.asarray(W1, dtype=np.float32))
    b1 = np.ascontiguousarray(np.asarray(b1, dtype=np.float32))
    W2 = np.ascontiguousarray(np.asarray(W2, dtype=np.float32))
    b2 = np.ascontiguousarray(np.asarray(b2, dtype=np.float32))

    if _NC_CACHE is None:
        _NC_CACHE = build_bass()
    nc = _NC_CACHE

    in_maps = []
    for i in range(N_CORES):
        in_maps.append(
            {
                "h": h[i * M_PER_CORE : (i + 1) * M_PER_CORE],
                "W1": W1,
                "b1": b1,
                "W2": W2,
                "b2": b2,
            }
        )

    res = run_bass_kernel_spmd(
        nc, in_maps, core_ids=list(range(N_CORES)), trace=_trace
    )
    _LAST_RESULTS = res
    out = np.concatenate([np.asarray(r["out"]) for r in res.results], axis=0)
    return out
